# revision 13
# baseline (speedup 1.0000x reference)
"""Trainium2 Bass kernel for nn_CPDP_AM_net_SGBS (3-layer MHA decoder step).

Contract: kernel(**inputs) takes FULL inputs (B=256) and returns the FULL
output (256, 512).  Internally shards the batch dim across 8 NeuronCores
(32 batches/core), data-parallel, no cross-core communication.

Per-core dataflow (b = 32 local batches, N=512 nodes, D=512, 16 heads of 32):
  layers 0,1 (16-head MHA + W0 projection), layer 2 (1-head scores -> output).
  - K_l[b] streamed HBM->SBUF [n,d], transposed to [d,n] on the PE
    (16x 128x128 transposes / batch-layer), PSUM->SBUF evacuated on DVE/ACT.
  - scores: lhsT = block-diag query cols (M=32: 16 head rows + 16 zero rows),
    rhs = K^T chunks, accumulated over 4 d-chunks into a [128,512] PSUM tile
    holding 4 batches at 32-row slots.  Mask folded in as one extra matmul
    (+= -1e9*mask broadcast through a 0/1 selector).
  - softmax: DVE negated-max, ACT exp(bias=-max) with fused row-sum
    (accum_out), DVE reciprocal + per-partition scale.
  - AV: w transposed on PE, then lhsT = w^T slot cols, rhs = V chunks.
  - head-diagonal extraction: Y*SELBIG (zero non-diagonal) then per-batch
    partition-sum matmuls against a 0/1 column -> attn^T [d,b] directly.
  - projections (W0, Wq) as 16 accumulated matmuls with transposed+scaled
    weights prepared on the host (scale folds the 1/sqrt(d_head) of the next
    attention layer).
  - layer 2: M=32 scores with zero-padded qf columns, 10*tanh + mask add,
    masked softmax, output rows DMA'd straight to DRAM.
"""

import sys

if "/opt/trn_rl_repo" not in sys.path:
    sys.path.insert(0, "/opt/trn_rl_repo")

import numpy as np

import concourse.bass as bass
import concourse.tile as tile
import concourse.mybir as mybir

F32 = mybir.dt.float32
F32R = mybir.dt.float32r
BF16 = mybir.dt.bfloat16

N_CORES = 8
B = 256
N = 512
D = 512
H = 16
DH = 32
NB_CHUNK = 4          # n chunks of 128
DC = 4                # d chunks of 128
CLIP = 10.0

USE_F32R = True       # fast path for the big N=512 matmul streams


def _r(ap):
    """dtype now carried by tile declarations; kept for call-site clarity."""
    return ap


def _hoist_excess_matmul_waits(nc, keep=1):
    """walrus limits self-loading 4-byte matmuls (fp32/fp32r/transpose) to a
    single sync wait on the S3_LW struct.  Hoist excess waits onto a
    standalone PE EventSemaphore inserted right before the matmul — same
    engine, so per-engine program order makes it equivalent."""
    for fn in nc.m.functions:
        for blk in fn.blocks:
            il = blk.instructions
            i = 0
            while i < len(il):
                inst = il[i]
                si = inst.sync_info
                if (type(inst).__name__ != "InstEventSemaphore"
                        and si is not None
                        and si.on_wait and len(si.on_wait) > keep):
                    moved = list(si.on_wait[:-keep]) if keep else list(si.on_wait)
                    kept = list(si.on_wait[-keep:]) if keep else []
                    for j, w in enumerate(moved):
                        wi = mybir.InstEventSemaphore(
                            name=f"{inst.name}-hw{j}",
                            ins=[], outs=[],
                            sync_info=mybir.SyncInfo(on_wait=[w], on_update=[]),
                        )
                        wi.engine = inst.engine
                        nc.register_instruction(wi)
                        il.insert(i, wi)
                        i += 1
                    inst.sync_info = mybir.SyncInfo(
                        on_wait=kept, on_update=list(si.on_update)
                    )
                i += 1


def build_nc(b_core=32):
    """Build the single-core Bass program for a [b_core]-batch shard."""
    groups = b_core // 4
    nc = bass.Bass()

    K_att = nc.declare_dram_parameter("K_att", [b_core, N, 3 * D], F32, isOutput=False)
    V_att = nc.declare_dram_parameter("V_att", [b_core, N, 3 * D], F32, isOutput=False)
    qbd0 = nc.declare_dram_parameter("qbd0", [128, DC, b_core * 32], BF16, isOutput=False)
    w0t = nc.declare_dram_parameter("w0t", [128, DC, D], F32, isOutput=False)
    wqt = nc.declare_dram_parameter("wqt", [128, DC, D], F32, isOutput=False)
    b0t = nc.declare_dram_parameter("b0t", [128, DC], F32, isOutput=False)
    bqt = nc.declare_dram_parameter("bqt", [128, DC], F32, isOutput=False)
    mb01 = nc.declare_dram_parameter("mb01", [4, groups, N], BF16, isOutput=False)
    mb2 = nc.declare_dram_parameter("mb2", [groups, 128, N], F32, isOutput=False)
    selbigt4 = nc.declare_dram_parameter("selbigt4", [128, DC, 32], F32, isOutput=False)
    msel = nc.declare_dram_parameter("msel", [4, 128], BF16, isOutput=False)
    ident = nc.declare_dram_parameter("ident", [128, 128], F32, isOutput=False)
    out = nc.declare_dram_parameter("out", [b_core, N], F32, isOutput=True)

    with tile.TileContext(nc) as tc:
        with (
            tc.tile_pool(name="singles", bufs=1) as singles,
            tc.tile_pool(name="kpool", bufs=3) as kpool,
            tc.tile_pool(name="vpool", bufs=3) as vpool,
            tc.tile_pool(name="ktpool", bufs=2) as ktpool,
            tc.tile_pool(name="work", bufs=2) as work,
            tc.tile_pool(name="small", bufs=4) as small,
            tc.tile_pool(name="p_kt", bufs=2, space="PSUM") as p_kt,
            tc.tile_pool(name="p_s", bufs=2, space="PSUM") as p_s,
            tc.tile_pool(name="p_wt", bufs=1, space="PSUM") as p_wt,
            tc.tile_pool(name="p_yt", bufs=2, space="PSUM") as p_yt,
            tc.tile_pool(name="p_q", bufs=1, space="PSUM") as p_q,
        ):
            # ---- constants / weights ----
            sb_qbd = singles.tile([128, DC, b_core * 32], BF16)
            nc.sync.dma_start(sb_qbd[:], qbd0[:])
            sb_w0t = singles.tile([128, DC, D], F32)
            nc.sync.dma_start(sb_w0t[:], w0t[:])
            sb_wqt = singles.tile([128, DC, D], F32)
            nc.sync.dma_start(sb_wqt[:], wqt[:])
            sb_b0t = singles.tile([128, DC], F32)
            nc.sync.dma_start(sb_b0t[:], b0t[:])
            sb_bqt = singles.tile([128, DC], F32)
            nc.sync.dma_start(sb_bqt[:], bqt[:])
            sb_mb01 = singles.tile([4, groups, N], BF16)
            nc.sync.dma_start(sb_mb01[:], mb01[:])
            sb_selbigt4 = singles.tile([128, DC, 32], F32)
            nc.sync.dma_start(sb_selbigt4[:], selbigt4[:])
            sb_msel = singles.tile([4, 128], BF16)
            nc.sync.dma_start(sb_msel[:], msel[:])
            sb_ident = singles.tile([128, 128], F32)
            nc.sync.dma_start(sb_ident[:], ident[:])

            def load_and_transpose_k(b, l, out_dt):
                """HBM K_l[b] -> SBUF [n,d] -> PE transpose -> SBUF K^T [d, c, n].

                out_dt=BF16 (layers 0/1): K^T downcast during the PSUM->SBUF
                copy; the transpose itself runs in f32r (1.5 cyc/row, rounding
                subsumed by the bf16 downcast).  out_dt=F32 (layer 2): exact."""
                ktile = kpool.tile([128, NB_CHUNK, D], F32, tag="ktile")
                nc.sync.dma_start(
                    ktile[:],
                    K_att[b, :, l * D:(l + 1) * D].rearrange("(c p) d -> p c d", p=128),
                )
                tag = "ktsb_b" if out_dt == BF16 else "ktsb_f"
                ktsb = ktpool.tile([128, DC, NB_CHUNK, 128], out_dt, tag=tag)
                for e in range(DC):
                    pkt = p_kt.tile([128, NB_CHUNK, 128], F32, tag="pkt")
                    for c in range(NB_CHUNK):
                        nc.tensor.transpose(
                            pkt[:, c, :], ktile[:, c, 128 * e:128 * e + 128], sb_ident[:]
                        )
                    # one producer engine per ktsb tile keeps the consuming
                    # matmul's sync-wait count within the S3_LW slot limit;
                    # alternate per batch to split load between DVE and ACT
                    if b % 2 == 0:
                        nc.vector.tensor_copy(ktsb[:, e, :, :], pkt[:])
                    else:
                        nc.scalar.copy(ktsb[:, e, :, :], pkt[:])
                return ktsb

            def softmax_weights(ps_s):
                """psum scores [128,512] -> normalized w [128,512] sbuf."""
                nmax = small.tile([128, 1], F32, tag="nmax")
                nc.vector.tensor_reduce(
                    nmax[:], ps_s[:], axis=mybir.AxisListType.X,
                    op=mybir.AluOpType.max, negate=True,
                )
                e_t = work.tile([128, N], F32, tag="e_t")
                zsum = small.tile([128, 1], F32, tag="zsum")
                nc.scalar.activation(
                    e_t[:], ps_s[:], mybir.ActivationFunctionType.Exp,
                    bias=nmax[:], scale=1.0, accum_out=zsum[:],
                )
                rz = small.tile([128, 1], F32, tag="rz")
                nc.vector.reciprocal(rz[:], zsum[:])
                w_t = work.tile([128, N], F32, tag="w_t")
                nc.vector.tensor_scalar_mul(w_t[:], e_t[:], rz[:])
                return w_t

            def projection(attn_sb, wt, bt, tag):
                """q_nextT [128, DC(j), b_core] = W^T @ attn^T + bias."""
                ps_q = p_q.tile([128, DC, b_core], F32, tag="ps_q")
                for jc in range(DC):
                    for ic in range(DC):
                        nc.tensor.matmul(
                            ps_q[:, jc, :],
                            wt[:, ic, 128 * jc:128 * jc + 128],
                            attn_sb[:, ic, :],
                            start=(ic == 0), stop=(ic == DC - 1),
                        )
                qt = work.tile([128, DC, b_core], F32, tag=tag)
                for jc in range(DC):
                    nc.vector.tensor_scalar_add(
                        qt[:, jc, :], ps_q[:, jc, :], bt[:, jc:jc + 1]
                    )
                return qt

            def fill_qbd_diag(qt):
                """Overwrite the block-diagonal of sb_qbd from qt [128, DC, b]."""
                qbd_v = sb_qbd.rearrange("p e (b j) -> p e b j", j=32)
                for e in range(DC):
                    for g in range(4):
                        nc.vector.tensor_copy(
                            qbd_v[32 * g:32 * g + 32, e, :, 4 * e + g],
                            qt[32 * g:32 * g + 32, e, :],
                        )

            # ================= layers 0, 1 =================
            qt_cur = None
            for l in range(2):
                if l > 0:
                    fill_qbd_diag(qt_cur)
                attn_sb = work.tile([128, DC, b_core], F32, tag="attn_sb")
                for g in range(groups):
                    ps_s = p_s.tile([128, N], F32, tag="ps_s")
                    for k in range(4):
                        b = 4 * g + k
                        ktsb = load_and_transpose_k(b, l, BF16)
                        for e in range(DC):
                            nc.tensor.matmul(
                                ps_s[32 * k:32 * k + 32, :],
                                sb_qbd[:, e, 32 * b:32 * b + 32],
                                ktsb[:, e, :, :],
                                start=(e == 0), stop=(e == DC - 1),
                                tile_position=(0, 32 * k),
                            )
                    nc.tensor.matmul(
                        ps_s[:],
                        sb_msel[:],
                        sb_mb01[:, g, :],
                        start=False, stop=True, skip_group_check=True,
                    )
                    w_t = softmax_weights(ps_s)
                    # w^T via PE
                    pwt = p_wt.tile([128, NB_CHUNK, 128], F32, tag="pwt")
                    for c in range(NB_CHUNK):
                        nc.tensor.transpose(
                            pwt[:, c, :], w_t[:, 128 * c:128 * c + 128], sb_ident[:]
                        )
                    wtsb = work.tile([128, NB_CHUNK, 128], F32, tag="wtsb")
                    nc.vector.tensor_copy(wtsb[:], pwt[:])
                    # AV flipped: V stationary, outputs Y^T [d, (slot,h)] at
                    # base partition 0 (fp32 exact, N=32)
                    for k in range(4):
                        b = 4 * g + k
                        vtile = vpool.tile([128, NB_CHUNK, D], F32, tag="vtile")
                        nc.sync.dma_start(
                            vtile[:],
                            V_att[b, :, l * D:(l + 1) * D].rearrange(
                                "(c p) d -> p c d", p=128
                            ),
                        )
                        ps_yt = p_yt.tile([128, DC, 32], F32, tag="ps_yt")
                        for dcc in range(DC):
                            for c in range(NB_CHUNK):
                                nc.tensor.matmul(
                                    ps_yt[:, dcc, :],
                                    vtile[:, c, 128 * dcc:128 * dcc + 128],
                                    wtsb[:, c, 32 * k:32 * k + 32],
                                    start=(c == 0), stop=(c == NB_CHUNK - 1),
                                )
                        # zero non-head-diagonal cols, then row-sum over the
                        # 32 head cols -> attn^T[:, dc] for this batch
                        zt = work.tile([128, DC, 32], F32, tag="zt")
                        nc.vector.tensor_mul(zt[:], ps_yt[:], sb_selbigt4[:])
                        nc.vector.tensor_reduce(
                            attn_sb[:, :, b], zt[:],
                            axis=mybir.AxisListType.X, op=mybir.AluOpType.add,
                        )
                if l == 0:
                    qt_cur = projection(attn_sb, sb_w0t, sb_b0t, "qt1")
                else:
                    q2t = projection(attn_sb, sb_w0t, sb_b0t, "qt2")
                    qt_cur = projection(q2t, sb_wqt, sb_bqt, "qft")

            # ================= layer 2 =================
            # zero-padded fp32 qf columns (col 32b = qf_b, rest zero)
            qf_pad = singles.tile([128, DC, b_core * 32], F32)
            nc.vector.memset(qf_pad[:], 0.0)
            qf_v = qf_pad.rearrange("p e (b j) -> p e b j", j=32)
            for e in range(DC):
                nc.vector.tensor_copy(qf_v[:, e, :, 0], qt_cur[:, e, :])
            for g in range(groups):
                ps_s2 = p_s.tile([128, N], F32, tag="ps_s")
                for k in range(4):
                    b = 4 * g + k
                    ktsb = load_and_transpose_k(b, 2, F32)
                    for e in range(DC):
                        nc.tensor.matmul(
                            ps_s2[32 * k:32 * k + 32, :],
                            qf_pad[:, e, 32 * b:32 * b + 32],
                            ktsb[:, e, :, :],
                            start=(e == 0), stop=(e == DC - 1),
                            tile_position=(0, 32 * k),
                        )
                # u = tanh(s2); v = u + (-1e8 * mask); e2 = exp(10*v - 10*max(v))
                u_t = work.tile([128, N], F32, tag="u_t")
                nc.scalar.activation(
                    u_t[:], ps_s2[:], mybir.ActivationFunctionType.Tanh
                )
                mb2t = work.tile([128, N], F32, tag="mb2t")
                nc.sync.dma_start(mb2t[:], mb2[g, :, :])
                v_t = work.tile([128, N], F32, tag="v_t")
                nc.vector.tensor_add(v_t[:], u_t[:], mb2t[:])
                nmax2 = small.tile([128, 1], F32, tag="nmax2")
                nc.vector.tensor_reduce(
                    nmax2[:], v_t[:], axis=mybir.AxisListType.X,
                    op=mybir.AluOpType.max, negate=True,
                )
                bias2 = small.tile([128, 1], F32, tag="bias2")
                nc.vector.tensor_scalar_mul(bias2[:], nmax2[:], CLIP)
                e2_t = work.tile([128, N], F32, tag="e2_t")
                zsum2 = small.tile([128, 1], F32, tag="zsum2")
                nc.scalar.activation(
                    e2_t[:], v_t[:], mybir.ActivationFunctionType.Exp,
                    bias=bias2[:], scale=CLIP, accum_out=zsum2[:],
                )
                rz2 = small.tile([128, 1], F32, tag="rz2")
                nc.vector.reciprocal(rz2[:], zsum2[:])
                w2_t = work.tile([128, N], F32, tag="w2_t")
                nc.vector.tensor_scalar_mul(w2_t[:], e2_t[:], rz2[:])
                nc.sync.dma_start(
                    out[4 * g:4 * g + 4, :],
                    w2_t.rearrange("(k r) n -> k r n", r=32)[:, 0, :],
                )
    _hoist_excess_matmul_waits(nc)
    return nc


# ---------------- host-side preparation ----------------

def _host_constants():
    import ml_dtypes
    p = np.arange(128)
    # selbigt4[p, dc, j] = 1 iff j == 4*dc + p//32  (the head owning row p of
    # Y^T chunk dc); zeroes both cross-head terms and the 16 garbage cols
    selbigt4 = np.zeros((128, DC, 32), np.float32)
    for dc in range(DC):
        selbigt4[np.arange(128), dc, 4 * dc + p // 32] = 1.0
    r = np.arange(4)
    msel = (((p // 32)[None, :] == r[:, None]) & ((p % 32) < 16)[None, :]
            ).astype(ml_dtypes.bfloat16)
    ident = np.eye(128, dtype=np.float32)
    return selbigt4, msel, ident


def _prep_core(query_c, mask_c, b_core):
    """Per-core block-diag query + mask bias tensors."""
    groups = b_core // 4
    qs = (query_c[:, 0, :] / np.sqrt(DH)).astype(np.float32)   # [b, D]
    qbd = np.zeros((128, DC, b_core, 32), np.float32)
    for e in range(DC):
        for g in range(4):
            # rows 32g..32g+32 of chunk e hold d = 128e + 32g .., head 4e+g
            qbd[32 * g:32 * g + 32, e, :, 4 * e + g] = qs[:, 128 * e + 32 * g:
                                                          128 * e + 32 * g + 32].T
    import ml_dtypes
    qbd = qbd.reshape(128, DC, b_core * 32).astype(ml_dtypes.bfloat16)

    mf = mask_c.astype(np.float32)                              # [b, N]
    mb01 = np.ascontiguousarray(
        -1e9 * mf.reshape(groups, 4, N).transpose(1, 0, 2)
    ).astype(ml_dtypes.bfloat16)
    mb2 = np.zeros((groups, 128, N), np.float32)
    for k in range(4):
        mb2[:, 32 * k, :] = -1e8 * mf.reshape(groups, 4, N)[:, k, :]
    return qbd, mb01, mb2


def _prep_weights(W0_w, W0_b, Wq_w, Wq_b):
    s0 = 1.0 / np.sqrt(DH)
    sq = np.sqrt(DH) / np.sqrt(D)
    w0t = (np.asarray(W0_w, np.float32).T * s0).reshape(DC, 128, D)
    w0t = np.ascontiguousarray(w0t.transpose(1, 0, 2))
    wqt = (np.asarray(Wq_w, np.float32).T * sq).reshape(DC, 128, D)
    wqt = np.ascontiguousarray(wqt.transpose(1, 0, 2))
    b0t = np.ascontiguousarray((np.asarray(W0_b, np.float32) * s0).reshape(DC, 128).T)
    bqt = np.ascontiguousarray((np.asarray(Wq_b, np.float32) / np.sqrt(D)).reshape(DC, 128).T)
    return w0t, wqt, b0t, bqt


_NC_CACHE = {}
TRACE = False          # test-harness hook: profile the run, fill LAST
LAST = {}


def kernel(query, K_att, V_att, mask, W0_w, W0_b, Wq_w, Wq_b):
    from concourse.bass_utils import run_bass_kernel_spmd

    query = np.asarray(query, np.float32)
    K_att = np.asarray(K_att, np.float32)
    V_att = np.asarray(V_att, np.float32)
    mask = np.asarray(mask)
    b_core = B // N_CORES

    if b_core not in _NC_CACHE:
        _NC_CACHE[b_core] = build_nc(b_core)
    nc = _NC_CACHE[b_core]

    selbigt4, msel, ident = _host_constants()
    w0t, wqt, b0t, bqt = _prep_weights(W0_w, W0_b, Wq_w, Wq_b)

    in_maps = []
    for i in range(N_CORES):
        sl = slice(i * b_core, (i + 1) * b_core)
        qbd, mb01, mb2 = _prep_core(query[sl], mask[sl], b_core)
        in_maps.append({
            "K_att": K_att[sl],
            "V_att": V_att[sl],
            "qbd0": qbd,
            "w0t": w0t, "wqt": wqt, "b0t": b0t, "bqt": bqt,
            "mb01": mb01, "mb2": mb2,
            "selbigt4": selbigt4, "msel": msel, "ident": ident,
        })

    rr = run_bass_kernel_spmd(nc, in_maps, list(range(N_CORES)), trace=TRACE)
    LAST["exec_time_ns"] = rr.exec_time_ns
    res = rr.results
    return np.concatenate([res[i]["out"] for i in range(N_CORES)], axis=0)
